# revision 8
# baseline (speedup 1.0000x reference)
"""Trainium2 Bass kernel for BoostedPointPairNet2 (v6).

Model (per (b, d) group, m = 128 points, din = 3):
  h1(i,j) = relu(u_j + v_i)            (64)   u_j = W1A x_j + b1, v_i = W1B x_i
  h2(i,j) = relu(W2 @ h1 + b2)         (128)
  G(i,j)  = W3 @ h2                    (256, b3 deferred)
  P       = max_{i,j} G + b3           (256)
  Y       = V3 @ relu(V2 @ relu(V1 @ P + c1) + c2) + c3  (40)
  out[b]  = max_d Y[b, d]

Sharding: 16 (b, d) groups over 8 cores, 2 groups per core; host does the
final max over d.

Design (v6 = v4's PE-side pre-H1 + v5's bank rotation and drain mix):
 * pre-H1 is computed ON THE PE via selection-matrix matmuls
     pre = uT.T @ sel  +  v2T.T @ iden      (K=64 + K=128, accumulated)
   into a SINGLE psum bank; b1 - b2 is folded into uT via an extra ones
   row of the x input so the fused relu's +b2 bias cancels exactly.
 * The h1 relu and h2 relu run FUSED as one [1536]-col ACT activation
   (+b2 bias): PSUM cols [pre 512 | l2A 512 | l2B 512] are contiguous,
   out combo = [h1(i+2) bf16 | h2(i) bf16].  One pre bank suffices: the
   sel/iden for epoch i+2 and the fused relu of epoch i both run in
   epoch i, and the next pre write is two epochs later.
 * L2 runs as TWO CONCURRENT row-tiled K=64 matmuls (W2 at array rows
   0:64 / 64:128, h1 parity halves as rhs) -> ~512 PE cycles.
 * G rotates over FIVE [128,512] psum banks (cols 1536:4096): epoch i
   uses banks (4i+k)%5, k=0..3; slot A (ch 0:128, w3a) = banks k=0,1;
   slot B (ch 128:256, w3b) = k=2,3.  The v4 L3->drain->L3 in-place
   recycle chain is gone; wrapped (non-contiguous) slots are drained as
   two [512] ops.
 * Drains: slot A -> DVE reduce_max into racc (P-half 0).  Slot B: on
   half the epochs ACT copies to fp16 + a LAGGED DVE tensor_tensor max
   into a ping-ponged running buffer (P-half 1); on the others a DVE
   reduce into racc2.  This balances ACT (relu+copies) against DVE
   (reduces+TTs) at ~2.1 us/epoch each.
 * F-MLP tail batched across groups (N=2 matmuls) with c1/c2/c3 folded
   in as K=1 matmul rows; vblob's DMA is issued last so the critical
   xt/bigb/sel DMAs land first.
"""

import numpy as np
import ml_dtypes

import bass_rust
import concourse.bass as bass
import concourse.mybir as mybir
from concourse.tile import TileContext
from concourse.bass_utils import run_bass_kernel_spmd

BF16 = ml_dtypes.bfloat16
F32 = np.float32
DT = mybir.dt
ALU = mybir.AluOpType
AX = mybir.AxisListType
RELU = mybir.ActivationFunctionType.Relu

N_CORES = 8
B, N, DIN = 4, 512, 3
D = 4                    # boost factor
M = N // D               # 128 points per group
GROUPS_PER_CORE = 2
JP = M // 2              # 64 stacked j-pairs per group
NITER = 32               # lockstep epochs (16 per group, 4 j's each)
NJUNK = 24               # junk matmuls to occupy PE while input DMAs land


def _is_copy(i):
    """Epochs whose G slot B takes the ACT-copy path (vs DVE dir-reduce)."""
    return (i // 2) % 2 == 0


def _split_multi_waits(nc):
    """This walrus build accepts at most ONE sync wait per instruction;
    hoist extra waits onto same-engine nops inserted before the offender."""
    seq = 0
    for fn in nc.m.functions:
        for bb in fn.blocks:
            new = []
            changed = False
            for ins in bb.instructions:
                si = ins.sync_info
                waits = list(si.on_wait) if si is not None and si.on_wait else []
                if len(waits) > 1:
                    changed = True
                    for w in waits[:-1]:
                        seq += 1
                        new.append(
                            mybir.InstNoOp(
                                name=f"I-wsplit-{seq}",
                                engine=ins.engine,
                                sync_info=bass_rust.SyncInfo(
                                    on_wait=[w], on_update=[]
                                ),
                            )
                        )
                    ins.sync_info = bass_rust.SyncInfo(
                        on_wait=[waits[-1]], on_update=list(si.on_update or [])
                    )
                new.append(ins)
            if changed:
                bb.instructions = new


# ---------------------------------------------------------------------------
# Device program
# ---------------------------------------------------------------------------
def _build_program():
    nc = bass.Bass(
        "TRN2", target_bir_lowering=False, debug=False, num_devices=N_CORES
    )

    # x with an appended ones row (folds b1 - b2 into the uT prep matmul)
    xt = nc.declare_dram_parameter(
        "xt", [DIN + 1, GROUPS_PER_CORE, M], DT.bfloat16, isOutput=False
    )
    # cols 0:64 urhs_e0, 64:128 urhs_e1, 128:256 w1b2, 256:768 iden,
    # 768:896 w2lo(rows 0:64), 896:1024 w2hi(rows 64:128),
    # 1024:1152 w3a, 1152:1280 w3b
    bigb = nc.declare_dram_parameter("bigb", [128, 1280], DT.bfloat16, isOutput=False)
    # sel[jp, it*512 + q*128 + i] = (jp == 4*it + q)
    selb = nc.declare_dram_parameter("selb", [JP, (NITER // 2) * 512], DT.bfloat16, isOutput=False)
    # v1t (2x512) | v2t (4x256) | v3t (2x40) | crows
    vblob = nc.declare_dram_parameter("vblob", [128, 2938], DT.float16, isOutput=False)
    # col 1 b2c, 2:4 b3_2, 4:12 c1_42, 12:16 c2_22, 16 c3
    cblob = nc.declare_dram_parameter("cblob", [128, 19], DT.float32, isOutput=False)
    y_out = nc.declare_dram_parameter(
        "y", [40, GROUPS_PER_CORE], DT.float32, isOutput=True
    )

    with TileContext(nc) as tc:
        with (
            tc.tile_pool(name="singles", bufs=1) as singles,
            tc.tile_pool(name="pers", bufs=1) as pers,
            tc.tile_pool(name="h2p", bufs=3) as h2pool,
            tc.tile_pool(name="gcp", bufs=3) as gcpool,
            tc.tile_pool(name="psum", bufs=1, space="PSUM") as psum,
        ):
            # The whole PSUM as one tile.
            # cols 0:512 pre | 512:1536 l2 (A,B) | 1536:4096 five G banks
            mega = psum.tile([128, 4096], DT.float32, tag="mega")

            # ---- input DMAs: critical (xt, bigb, sel) first; vblob last ----
            sb_xtall = singles.tile([DIN + 1, GROUPS_PER_CORE, M], DT.bfloat16, tag="xtall")
            nc.sync.dma_start(out=sb_xtall, in_=xt[:, :, :])
            sb_xts = [sb_xtall[:, 0, :], sb_xtall[:, 1, :]]
            sb_big = singles.tile([128, 1280], DT.bfloat16, tag="bigb")
            nc.sync.dma_start(out=sb_big, in_=bigb[:, :])
            sb_sel = singles.tile([JP, (NITER // 2) * 512], DT.bfloat16, tag="selb")
            nc.sync.dma_start(out=sb_sel[:, 0:2048], in_=selb[:, 0:2048])
            nc.sync.dma_start(out=sb_sel[:, 2048:5120], in_=selb[:, 2048:5120])
            sb_c = singles.tile([128, 19], DT.float32, tag="cblob")
            nc.gpsimd.dma_start(out=sb_c, in_=cblob[:, :])
            nc.gpsimd.dma_start(out=sb_sel[:, 5120:8192], in_=selb[:, 5120:8192])
            sb_v = singles.tile([128, 2938], DT.float16, tag="vblob")
            nc.gpsimd.dma_start(out=sb_v, in_=vblob[:, :])

            sb_urhs0 = sb_big[0 : DIN + 1, 0:64]
            sb_urhs1 = sb_big[0 : DIN + 1, 64:128]
            sb_w1b2 = sb_big[0:DIN, 128:256]
            sb_iden = sb_big[:, 256:768]
            w2lo = sb_big[0:64, 768:896]
            w2hi = sb_big[64:128, 896:1024]
            w3a = sb_big[:, 1024:1152]
            w3b = sb_big[:, 1152:1280]

            # dummy relu to hoist ACT_TABLE_LOAD into the init shadow
            warm = singles.tile([1, 1], DT.float32, tag="warm")
            nc.vector.memset(warm, 0.0)
            nc.scalar.activation(out=warm, in_=warm, func=RELU)

            # ---- junk matmuls: keep PE busy while DMAs land ----
            wjunk = singles.tile([128, 128], DT.bfloat16, tag="wjunk")
            nc.vector.memset(wjunk, 0.0)
            for _ in range(NJUNK):
                nc.tensor.matmul(
                    mega[:, 3584:3712], lhsT=wjunk, rhs=wjunk,
                    start=True, stop=True,
                )

            sb_b2c = sb_c[:, 1:2]
            sb_b3_2 = sb_c[:, 2:4]

            def v1t(k):  # [128, 512] fp16, k in 0..1
                return sb_v[:, 512 * k : 512 * (k + 1)]

            def v2t(k):  # [128, 256] fp16, k in 0..3
                return sb_v[:, 1024 + 256 * k : 1024 + 256 * (k + 1)]

            def v3t(k):  # [128, 40] fp16, k in 0..1
                return sb_v[:, 2048 + 40 * k : 2048 + 40 * (k + 1)]

            # ---- per-group prep: uT [64,128] and v2T [128,128] in SBUF bf16
            # (psum carved from the G region, consumed before first L3) ----
            uT_sbs, v2T_sbs = [], []
            for g in range(GROUPS_PER_CORE):
                sb_xt = sb_xts[g]
                xt_eo = sb_xt.rearrange("k (j two) -> k two j", two=2)
                uTps = mega[0:JP, 1536 + 512 * g : 1536 + 512 * g + 128]
                nc.tensor.matmul(
                    uTps[:, 0:64], lhsT=xt_eo[:, 0, :], rhs=sb_urhs0,
                    start=True, stop=True,
                )
                nc.tensor.matmul(
                    uTps[:, 64:128], lhsT=xt_eo[:, 1, :], rhs=sb_urhs1,
                    start=True, stop=True,
                )
                uT_sb = pers.tile([JP, 128], DT.bfloat16, tag=f"uT{g}", name=f"uT{g}")
                nc.vector.tensor_copy(out=uT_sb, in_=uTps)
                v2Tps = mega[:, 2560 + 512 * g : 2560 + 512 * g + 128]
                nc.tensor.matmul(
                    v2Tps, lhsT=sb_xt[0:DIN, :], rhs=sb_w1b2,
                    start=True, stop=True,
                )
                v2T_sb = pers.tile([128, 128], DT.bfloat16, tag=f"v2T{g}", name=f"v2T{g}")
                nc.vector.tensor_copy(out=v2T_sb, in_=v2Tps)
                uT_sbs.append(uT_sb)
                v2T_sbs.append(v2T_sb)

            # per-group accumulators
            raccs, racc2s, pm2s = [], [], []
            for g in range(GROUPS_PER_CORE):
                raccs.append(pers.tile([128, 20], DT.float32, tag=f"racc{g}",
                                       name=f"racc{g}"))
                racc2s.append(pers.tile([128, 10], DT.float32, tag=f"racc2{g}",
                                        name=f"racc2{g}"))
                pm2s.append(pers.tile([128, 2], DT.float32, tag=f"pm2{g}",
                                      name=f"pm2{g}"))
            rb_bufs = [
                pers.tile([128, 1024], DT.float16, tag=f"rbb{k}", name=f"rbb{k}")
                for k in range(3)
            ]
            rb_cur = {0: None, 1: None}
            dcount = [0, 0]
            d2count = [0, 0]

            def issue_sel(i):
                """pre(i) = uT.T @ sel_it + v2T.T @ iden  (accumulated)."""
                g, it = i % 2, i // 2
                pre = mega[:, 0:512]
                nc.tensor.matmul(
                    pre, lhsT=uT_sbs[g],
                    rhs=sb_sel[:, 512 * it : 512 * (it + 1)],
                    start=True, stop=False,
                )
                nc.tensor.matmul(
                    pre, lhsT=v2T_sbs[g], rhs=sb_iden, start=False, stop=True,
                )

            def issue_relu():
                """Fused relu of [pre(i+2) | l2(i)] -> [h1(i+2) | h2(i)]."""
                combo = h2pool.tile([128, 1536], DT.bfloat16)
                nc.scalar.activation(
                    out=combo, in_=mega[:, 0:1536], func=RELU,
                    bias=sb_b2c, scale=1.0,
                )
                return combo

            def issue_l2(h1):
                nc.tensor.matmul(
                    mega[:, 512:1024], lhsT=w2lo, rhs=h1[0:64, :],
                    start=True, stop=True,
                )
                nc.tensor.matmul(
                    mega[:, 1024:1536], lhsT=w2hi, rhs=h1[64:128, :],
                    start=True, stop=True,
                )

            def g_bank(b):
                return mega[:, 1536 + 512 * b : 2048 + 512 * b]

            def g_banks(i):
                base = (4 * i) % 5
                return [(base + k) % 5 for k in range(4)]

            def issue_l3(i, h2):
                bk = g_banks(i)
                nc.tensor.matmul(
                    g_bank(bk[0]), lhsT=w3a, rhs=h2[:, 0:512],
                    start=True, stop=True,
                )
                nc.tensor.matmul(
                    g_bank(bk[1]), lhsT=w3a, rhs=h2[:, 512:1024],
                    start=True, stop=True,
                )
                nc.tensor.matmul(
                    g_bank(bk[2]), lhsT=w3b, rhs=h2[:, 0:512],
                    start=True, stop=True,
                )
                nc.tensor.matmul(
                    g_bank(bk[3]), lhsT=w3b, rhs=h2[:, 512:1024],
                    start=True, stop=True,
                )

            def dir_reduce(g, racc, cnt, b0, b1):
                """reduce_max of G banks b0,b1 into racc columns."""
                if b1 == b0 + 1:
                    t = cnt[g]
                    cnt[g] += 1
                    nc.vector.reduce_max(
                        out=racc[g][:, t : t + 1],
                        in_=mega[:, 1536 + 512 * b0 : 2560 + 512 * b0],
                        axis=AX.X,
                    )
                else:
                    for b in (b0, b1):
                        t = cnt[g]
                        cnt[g] += 1
                        nc.vector.reduce_max(
                            out=racc[g][:, t : t + 1], in_=g_bank(b), axis=AX.X
                        )

            def issue_drains(i):
                """Drain G(i): slot A (ch 0:128) -> racc; slot B: ACT copy
                (lagged TT) on copy epochs, else DVE reduce into racc2."""
                g = i % 2
                bk = g_banks(i)
                dir_reduce(g, raccs, dcount, bk[0], bk[1])
                if _is_copy(i):
                    gc = gcpool.tile([128, 1024], DT.float16)
                    if bk[3] == bk[2] + 1:
                        nc.scalar.copy(out=gc, in_=mega[:, 1536 + 512 * bk[2] : 2560 + 512 * bk[2]])
                    else:
                        nc.scalar.copy(out=gc[:, 0:512], in_=g_bank(bk[2]))
                        nc.scalar.copy(out=gc[:, 512:1024], in_=g_bank(bk[3]))
                    return gc
                dir_reduce(g, racc2s, d2count, bk[2], bk[3])
                return None

            def issue_tt(i, gc, spare):
                """Lagged running max of epoch i's slot-B copy."""
                g = i % 2
                if rb_cur[g] is None:
                    rb_cur[g] = rb_bufs[g]
                    nc.vector.tensor_copy(out=rb_cur[g], in_=gc)
                    return spare
                nc.vector.tensor_tensor(
                    out=spare, in0=gc, in1=rb_cur[g], op=ALU.max
                )
                old = rb_cur[g]
                rb_cur[g] = spare
                return old

            # ---- main lockstep pipeline: sel(i+2), fused relu(i), L2(i+1),
            # L3(i-1), drains(i-1), lagged TT(i-2) ----
            h1s, h2s, gcs = {}, {}, {}
            tt_spare = rb_bufs[2]

            issue_sel(0)
            c0 = h2pool.tile([128, 1536], DT.bfloat16)
            nc.scalar.activation(out=c0[:, 0:512], in_=mega[:, 0:512],
                                 func=RELU, bias=sb_b2c, scale=1.0)
            h1s[0] = c0[:, 0:512]
            issue_l2(h1s[0])
            issue_sel(1)
            c1t = h2pool.tile([128, 1536], DT.bfloat16)
            nc.scalar.activation(out=c1t[:, 0:512], in_=mega[:, 0:512],
                                 func=RELU, bias=sb_b2c, scale=1.0)
            h1s[1] = c1t[:, 0:512]

            for i in range(NITER):
                if i + 2 < NITER:
                    issue_sel(i + 2)
                    combo = issue_relu()
                    h1s[i + 2] = combo[:, 0:512]
                    h2s[i] = combo[:, 512:1536]
                else:
                    combo = h2pool.tile([128, 1536], DT.bfloat16)
                    nc.scalar.activation(
                        out=combo[:, 512:1536], in_=mega[:, 512:1536],
                        func=RELU, bias=sb_b2c, scale=1.0,
                    )
                    h2s[i] = combo[:, 512:1536]
                if i + 1 < NITER:
                    issue_l2(h1s[i + 1])
                if i >= 1:
                    issue_l3(i - 1, h2s[i - 1])
                    gc = issue_drains(i - 1)
                    if gc is not None:
                        gcs[i - 1] = gc
                if i >= 2 and (i - 2) in gcs:
                    tt_spare = issue_tt(i - 2, gcs.pop(i - 2), tt_spare)
            issue_l3(NITER - 1, h2s[NITER - 1])
            gc = issue_drains(NITER - 1)
            if gc is not None:
                gcs[NITER - 1] = gc
            for i in (NITER - 2, NITER - 1):
                if i in gcs:
                    tt_spare = issue_tt(i, gcs.pop(i), tt_spare)

            # ---- P per group, batched F-MLP (N=2); pb is (half, group) ----
            pb = pers.tile([128, 2, 2], DT.float16, tag="pb")
            tmp2s = pers.tile([128, 2], DT.float32, tag="tmp2s")
            for g in range(GROUPS_PER_CORE):
                nc.vector.reduce_max(
                    out=pm2s[g][:, 0:1], in_=raccs[g][:, 0 : dcount[g]],
                    axis=AX.X,
                )
                nc.vector.reduce_max(
                    out=pm2s[g][:, 1:2], in_=rb_cur[g], axis=AX.X
                )
                nc.vector.reduce_max(
                    out=tmp2s[:, g : g + 1], in_=racc2s[g][:, 0 : d2count[g]],
                    axis=AX.X,
                )
                nc.vector.tensor_tensor(
                    out=pm2s[g][:, 1:2], in0=pm2s[g][:, 1:2],
                    in1=tmp2s[:, g : g + 1], op=ALU.max,
                )
                nc.vector.tensor_tensor(
                    out=pb[:, :, g], in0=pm2s[g], in1=sb_b3_2, op=ALU.add
                )

            ones2 = sb_v[0:1, 2936:2938]
            y1ps = mega[:, 0:8].rearrange("p (m g) -> p m g", m=4)
            for mm in range(4):
                for kk in range(2):
                    nc.tensor.matmul(
                        y1ps[:, mm, :],
                        lhsT=v1t(kk)[:, mm * 128 : (mm + 1) * 128],
                        rhs=pb[:, kk, :],
                        start=(kk == 0),
                        stop=False,
                    )
                nc.tensor.matmul(
                    y1ps[:, mm, :],
                    lhsT=sb_v[0:1, 2128 + mm * 128 : 2128 + (mm + 1) * 128],
                    rhs=ones2,
                    start=False, stop=True,
                )
            y1 = pers.tile([128, 4, 2], DT.float16, tag="y1")
            nc.vector.tensor_scalar_max(
                out=y1.rearrange("p m g -> p (m g)"), in0=mega[:, 0:8],
                scalar1=0.0,
            )

            y2ps = mega[:, 1024:1028].rearrange("p (m g) -> p m g", m=2)
            for mm in range(2):
                for kk in range(4):
                    nc.tensor.matmul(
                        y2ps[:, mm, :],
                        lhsT=v2t(kk)[:, mm * 128 : (mm + 1) * 128],
                        rhs=y1[:, kk, :],
                        start=(kk == 0),
                        stop=False,
                    )
                nc.tensor.matmul(
                    y2ps[:, mm, :],
                    lhsT=sb_v[0:1, 2640 + mm * 128 : 2640 + (mm + 1) * 128],
                    rhs=ones2,
                    start=False, stop=True,
                )
            y2 = pers.tile([128, 2, 2], DT.float16, tag="y2")
            nc.vector.tensor_scalar_max(
                out=y2.rearrange("p m g -> p (m g)"), in0=mega[:, 1024:1028],
                scalar1=0.0,
            )

            y3ps = mega[0:40, 2048:2050]
            for kk in range(2):
                nc.tensor.matmul(
                    y3ps,
                    lhsT=v3t(kk)[:, 0:40],
                    rhs=y2[:, kk, :],
                    start=(kk == 0),
                    stop=False,
                )
            nc.tensor.matmul(
                y3ps, lhsT=sb_v[0:1, 2896:2936], rhs=ones2,
                start=False, stop=True,
            )
            y3 = pers.tile([40, 2], DT.float32, tag="y3")
            nc.vector.tensor_copy(out=y3, in_=y3ps)
            nc.gpsimd.dma_start(out=y_out[:, :], in_=y3)

    _split_multi_waits(nc)
    return nc


# ---------------------------------------------------------------------------
# Host side
# ---------------------------------------------------------------------------
_NC_CACHE = None


def _get_program():
    global _NC_CACHE
    if _NC_CACHE is None:
        _NC_CACHE = _build_program()
    return _NC_CACHE


def _make_in_maps(inputs):
    X = np.asarray(inputs["X"], F32)
    W1 = np.asarray(inputs["W1"], F32)
    b1 = np.asarray(inputs["b1"], F32)
    W2 = np.asarray(inputs["W2"], F32)
    b2 = np.asarray(inputs["b2"], F32)
    W3 = np.asarray(inputs["W3"], F32)
    b3 = np.asarray(inputs["b3"], F32)
    V1 = np.asarray(inputs["V1"], F32)
    c1 = np.asarray(inputs["c1"], F32)
    V2 = np.asarray(inputs["V2"], F32)
    c2 = np.asarray(inputs["c2"], F32)
    V3 = np.asarray(inputs["V3"], F32)
    c3 = np.asarray(inputs["c3"], F32)

    W1A, W1B = W1[:, :DIN], W1[:, DIN:]
    # sel[jp, it*512 + q*128 + i] = (jp == 4*it + q)
    selblob = np.zeros((JP, (NITER // 2) * 512), F32)
    for jp in range(JP):
        it, q = jp // 4, jp % 4
        selblob[jp, it * 512 + q * 128 : it * 512 + (q + 1) * 128] = 1.0
    selblob = selblob.astype(BF16)

    bigblob = np.zeros((128, 1280), F32)
    # u-rhs halves: ones-row contributes b1 - b2[half] (cancels the fused
    # relu's +b2 bias on the pre region)
    bigblob[0:DIN, 0:64] = W1A.T
    bigblob[DIN, 0:64] = b1 - b2[0:64]
    bigblob[0:DIN, 64:128] = W1A.T
    bigblob[DIN, 64:128] = b1 - b2[64:128]
    bigblob[0:DIN, 128:256] = np.concatenate([W1B.T, W1B.T], axis=1)
    # iden region
    bigblob[:, 256:768] = np.tile(np.eye(M, dtype=F32), (1, 4))
    # packed W2 + W3
    bigblob[0:64, 768:896] = W2.T
    bigblob[64:128, 896:1024] = W2.T
    bigblob[:, 1024:1152] = W3.T[:, 0:128]
    bigblob[:, 1152:1280] = W3.T[:, 128:256]
    bigblob = bigblob.astype(BF16)

    v1t_cols = V1.T.reshape(2, 128, 512).transpose(1, 0, 2).reshape(128, 1024)
    crows = np.zeros((128, 810), F32)
    crows[0, 0:512] = c1
    crows[0, 512:768] = c2
    crows[0, 768:808] = c3
    crows[0, 808:810] = 1.0
    vblob = np.concatenate(
        [v1t_cols,
         V2.T.reshape(4, 128, 256).transpose(1, 0, 2).reshape(128, 1024),
         V3.T.reshape(2, 128, 40).transpose(1, 0, 2).reshape(128, 80),
         crows],
        axis=1,
    ).astype(np.float16)
    cblob = np.zeros((128, 19), F32)
    cblob[:, 17:19] = 1.0
    cblob[:, 1] = b2
    cblob[:, 2:4] = b3.reshape(2, 128).T
    cblob[:, 4:12] = np.repeat(c1.reshape(4, 128).T, 2, axis=1)
    cblob[:, 12:16] = np.repeat(c2.reshape(2, 128).T, 2, axis=1)
    cblob[0:40, 16] = c3

    shared = dict(
        bigb=bigblob, vblob=vblob, cblob=cblob, selb=selblob,
    )

    Xv = X.reshape(B, D, M, DIN)
    in_maps = []
    for c in range(N_CORES):
        xts = np.ones((DIN + 1, GROUPS_PER_CORE, M), F32)
        for gi in range(GROUPS_PER_CORE):
            g = 2 * c + gi
            bb, dd = g // D, g % D
            xts[0:DIN, gi] = Xv[bb, dd].T
        in_maps.append(dict(shared, xt=xts.astype(BF16)))
    return in_maps


def _run(inputs, trace=False):
    nc = _get_program()
    in_maps = _make_in_maps(inputs)
    res = run_bass_kernel_spmd(nc, in_maps, list(range(N_CORES)), trace=trace)
    ys = np.stack([res.results[c]["y"].T for c in range(N_CORES)])  # [8, 2, 40]
    y16 = ys.reshape(B, D, 40)
    out = y16.max(axis=1).astype(F32)
    return out, res


def kernel(**inputs):
    out, _ = _run(inputs, trace=False)
    return out


# revision 10
# speedup vs baseline: 1.0717x; 1.0717x over previous
"""Trainium2 Bass kernel for BoostedPointPairNet2 (v6).

Model (per (b, d) group, m = 128 points, din = 3):
  h1(i,j) = relu(u_j + v_i)            (64)   u_j = W1A x_j + b1, v_i = W1B x_i
  h2(i,j) = relu(W2 @ h1 + b2)         (128)
  G(i,j)  = W3 @ h2                    (256, b3 deferred)
  P       = max_{i,j} G + b3           (256)
  Y       = V3 @ relu(V2 @ relu(V1 @ P + c1) + c2) + c3  (40)
  out[b]  = max_d Y[b, d]

Sharding: 16 (b, d) groups over 8 cores, 2 groups per core; host does the
final max over d.

Design (v6 = v4's PE-side pre-H1 + v5's bank rotation and drain mix):
 * pre-H1 is computed ON THE PE via selection-matrix matmuls
     pre = uT.T @ sel  +  v2T.T @ iden      (K=64 + K=128, accumulated)
   into a SINGLE psum bank; b1 - b2 is folded into uT via an extra ones
   row of the x input so the fused relu's +b2 bias cancels exactly.
 * The h1 relu and h2 relu run FUSED as one [1536]-col ACT activation
   (+b2 bias): PSUM cols [pre 512 | l2A 512 | l2B 512] are contiguous,
   out combo = [h1(i+2) bf16 | h2(i) bf16].  One pre bank suffices: the
   sel/iden for epoch i+2 and the fused relu of epoch i both run in
   epoch i, and the next pre write is two epochs later.
 * L2 runs as TWO CONCURRENT row-tiled K=64 matmuls (W2 at array rows
   0:64 / 64:128, h1 parity halves as rhs) -> ~512 PE cycles.
 * G rotates over FIVE [128,512] psum banks (cols 1536:4096): epoch i
   uses banks (4i+k)%5, k=0..3; slot A (ch 0:128, w3a) = banks k=0,1;
   slot B (ch 128:256, w3b) = k=2,3.  The v4 L3->drain->L3 in-place
   recycle chain is gone; wrapped (non-contiguous) slots are drained as
   two [512] ops.
 * Drains: slot A -> DVE reduce_max into racc (P-half 0).  Slot B: on
   half the epochs ACT copies to fp16 + a LAGGED DVE tensor_tensor max
   into a ping-ponged running buffer (P-half 1); on the others a DVE
   reduce into racc2.  This balances ACT (relu+copies) against DVE
   (reduces+TTs) at ~2.1 us/epoch each.
 * F-MLP tail batched across groups (N=2 matmuls) with c1/c2/c3 folded
   in as K=1 matmul rows; vblob's DMA is issued last so the critical
   xt/bigb/sel DMAs land first.
"""

import numpy as np
import ml_dtypes

import bass_rust
import concourse.bass as bass
import concourse.mybir as mybir
from concourse.tile import TileContext
from concourse.bass_utils import run_bass_kernel_spmd

BF16 = ml_dtypes.bfloat16
F32 = np.float32
DT = mybir.dt
ALU = mybir.AluOpType
AX = mybir.AxisListType
RELU = mybir.ActivationFunctionType.Relu

N_CORES = 8
B, N, DIN = 4, 512, 3
D = 4                    # boost factor
M = N // D               # 128 points per group
GROUPS_PER_CORE = 2
JP = M // 2              # 64 stacked j-pairs per group
NITER = 32               # lockstep epochs (16 per group, 4 j's each)
NJUNK = 12               # junk matmuls to occupy PE while input DMAs land


def _is_copy(i):
    """Epochs whose G slot B takes the ACT-copy path (vs DVE dir-reduce)."""
    return (i // 2) % 3 == 0


def _split_multi_waits(nc):
    """This walrus build accepts at most ONE sync wait per instruction;
    hoist extra waits onto same-engine nops inserted before the offender."""
    seq = 0
    for fn in nc.m.functions:
        for bb in fn.blocks:
            new = []
            changed = False
            for ins in bb.instructions:
                si = ins.sync_info
                waits = list(si.on_wait) if si is not None and si.on_wait else []
                if len(waits) > 1:
                    changed = True
                    for w in waits[:-1]:
                        seq += 1
                        new.append(
                            mybir.InstNoOp(
                                name=f"I-wsplit-{seq}",
                                engine=ins.engine,
                                sync_info=bass_rust.SyncInfo(
                                    on_wait=[w], on_update=[]
                                ),
                            )
                        )
                    ins.sync_info = bass_rust.SyncInfo(
                        on_wait=[waits[-1]], on_update=list(si.on_update or [])
                    )
                new.append(ins)
            if changed:
                bb.instructions = new


# ---------------------------------------------------------------------------
# Device program
# ---------------------------------------------------------------------------
def _build_program():
    nc = bass.Bass(
        "TRN2", target_bir_lowering=False, debug=False, num_devices=N_CORES
    )

    # x with an appended ones row (folds b1 - b2 into the uT prep matmul)
    xt = nc.declare_dram_parameter(
        "xt", [DIN + 1, GROUPS_PER_CORE, M], DT.bfloat16, isOutput=False
    )
    # cols 0:64 urhs_e0, 64:128 urhs_e1, 128:256 w1b2, 256:768 iden,
    # 768:896 w2lo(rows 0:64), 896:1024 w2hi(rows 64:128),
    # 1024:1152 w3a, 1152:1280 w3b
    bigb = nc.declare_dram_parameter("bigb", [128, 1280], DT.bfloat16, isOutput=False)
    # sel[jp, it*512 + q*128 + i] = (jp == 4*it + q)
    selb = nc.declare_dram_parameter("selb", [JP, (NITER // 2) * 512], DT.bfloat16, isOutput=False)
    # v1t (2x512) | v2t (4x256) | v3t (2x40) | crows
    vblob = nc.declare_dram_parameter("vblob", [128, 2938], DT.float16, isOutput=False)
    # col 1 b2c, 2:4 b3_2, 4:12 c1_42, 12:16 c2_22, 16 c3
    cblob = nc.declare_dram_parameter("cblob", [128, 19], DT.float32, isOutput=False)
    y_out = nc.declare_dram_parameter(
        "y", [40, GROUPS_PER_CORE], DT.float32, isOutput=True
    )

    with TileContext(nc) as tc:
        with (
            tc.tile_pool(name="singles", bufs=1) as singles,
            tc.tile_pool(name="pers", bufs=1) as pers,
            tc.tile_pool(name="h1p", bufs=4) as h1pool,
            tc.tile_pool(name="h2p", bufs=3) as h2pool,
            tc.tile_pool(name="gcp", bufs=3) as gcpool,
            tc.tile_pool(name="psum", bufs=1, space="PSUM") as psum,
        ):
            # The whole PSUM as one tile.
            # cols 0:512 pre | 512:1536 l2 (A,B) | 1536:4096 five G banks
            mega = psum.tile([128, 4096], DT.float32, tag="mega")

            # ---- input DMAs: critical (xt, bigb, sel) first; vblob last ----
            sb_xtall = singles.tile([DIN + 1, GROUPS_PER_CORE, M], DT.bfloat16, tag="xtall")
            nc.sync.dma_start(out=sb_xtall, in_=xt[:, :, :])
            sb_xts = [sb_xtall[:, 0, :], sb_xtall[:, 1, :]]
            sb_big = singles.tile([128, 1280], DT.bfloat16, tag="bigb")
            nc.sync.dma_start(out=sb_big, in_=bigb[:, :])
            sb_sel = singles.tile([JP, (NITER // 2) * 512], DT.bfloat16, tag="selb")
            nc.sync.dma_start(out=sb_sel[:, 0:2048], in_=selb[:, 0:2048])
            nc.sync.dma_start(out=sb_sel[:, 2048:5120], in_=selb[:, 2048:5120])
            sb_c = singles.tile([128, 19], DT.float32, tag="cblob")
            nc.gpsimd.dma_start(out=sb_c, in_=cblob[:, :])
            nc.gpsimd.dma_start(out=sb_sel[:, 5120:8192], in_=selb[:, 5120:8192])
            sb_v = singles.tile([128, 2938], DT.float16, tag="vblob")
            nc.sync.dma_start(out=sb_v, in_=vblob[:, :])

            sb_urhs0 = sb_big[0 : DIN + 1, 0:64]
            sb_urhs1 = sb_big[0 : DIN + 1, 64:128]
            sb_w1b2 = sb_big[0:DIN, 128:256]
            sb_iden = sb_big[:, 256:768]
            w2lo = sb_big[0:64, 768:896]
            w2hi = sb_big[64:128, 896:1024]
            w3a = sb_big[:, 1024:1152]
            w3b = sb_big[:, 1152:1280]

            # dummy relu to hoist ACT_TABLE_LOAD into the init shadow
            warm = singles.tile([1, 1], DT.float32, tag="warm")
            nc.vector.memset(warm, 0.0)
            nc.scalar.activation(out=warm, in_=warm, func=RELU)

            # ---- junk matmuls: keep PE busy while DMAs land ----
            wjunk = singles.tile([128, 128], DT.bfloat16, tag="wjunk")
            nc.vector.memset(wjunk, 0.0)
            for _ in range(NJUNK):
                nc.tensor.matmul(
                    mega[:, 3584:3712], lhsT=wjunk, rhs=wjunk,
                    start=True, stop=True,
                )

            sb_b2c = sb_c[:, 1:2]
            sb_b3_2 = sb_c[:, 2:4]

            def v1t(k):  # [128, 512] fp16, k in 0..1
                return sb_v[:, 512 * k : 512 * (k + 1)]

            def v2t(k):  # [128, 256] fp16, k in 0..3
                return sb_v[:, 1024 + 256 * k : 1024 + 256 * (k + 1)]

            def v3t(k):  # [128, 40] fp16, k in 0..1
                return sb_v[:, 2048 + 40 * k : 2048 + 40 * (k + 1)]

            # ---- per-group prep: uT [64,128] and v2T [128,128] in SBUF bf16
            # (psum carved from the G region, consumed before first L3) ----
            uT_sbs, v2T_sbs = [], []
            for g in range(GROUPS_PER_CORE):
                sb_xt = sb_xts[g]
                xt_eo = sb_xt.rearrange("k (j two) -> k two j", two=2)
                uTps = mega[0:JP, 1536 + 512 * g : 1536 + 512 * g + 128]
                nc.tensor.matmul(
                    uTps[:, 0:64], lhsT=xt_eo[:, 0, :], rhs=sb_urhs0,
                    start=True, stop=True,
                )
                nc.tensor.matmul(
                    uTps[:, 64:128], lhsT=xt_eo[:, 1, :], rhs=sb_urhs1,
                    start=True, stop=True,
                )
                uT_sb = pers.tile([JP, 128], DT.bfloat16, tag=f"uT{g}", name=f"uT{g}")
                nc.vector.tensor_copy(out=uT_sb, in_=uTps)
                v2Tps = mega[:, 2560 + 512 * g : 2560 + 512 * g + 128]
                nc.tensor.matmul(
                    v2Tps, lhsT=sb_xt[0:DIN, :], rhs=sb_w1b2,
                    start=True, stop=True,
                )
                v2T_sb = pers.tile([128, 128], DT.bfloat16, tag=f"v2T{g}", name=f"v2T{g}")
                nc.vector.tensor_copy(out=v2T_sb, in_=v2Tps)
                uT_sbs.append(uT_sb)
                v2T_sbs.append(v2T_sb)

            # per-group accumulators
            raccs, racc2s, pm2s = [], [], []
            for g in range(GROUPS_PER_CORE):
                raccs.append(pers.tile([128, 20], DT.float32, tag=f"racc{g}",
                                       name=f"racc{g}"))
                racc2s.append(pers.tile([128, 14], DT.float32, tag=f"racc2{g}",
                                        name=f"racc2{g}"))
                pm2s.append(pers.tile([128, 2], DT.float32, tag=f"pm2{g}",
                                      name=f"pm2{g}"))
            rb_bufs = [
                pers.tile([128, 1024], DT.float16, tag=f"rbb{k}", name=f"rbb{k}")
                for k in range(3)
            ]
            rb_cur = {0: None, 1: None}
            dcount = [0, 0]
            d2count = [0, 0]

            def issue_sel(i):
                """pre(i) = uT.T @ sel_it + v2T.T @ iden  (accumulated)."""
                g, it = i % 2, i // 2
                pre = mega[:, 0:512]
                nc.tensor.matmul(
                    pre, lhsT=uT_sbs[g],
                    rhs=sb_sel[:, 512 * it : 512 * (it + 1)],
                    start=True, stop=False,
                )
                nc.tensor.matmul(
                    pre, lhsT=v2T_sbs[g], rhs=sb_iden, start=False, stop=True,
                )

            def issue_prerelu():
                """relu of pre -> h1 in SBUF (gates only the next sel)."""
                h1 = h1pool.tile([128, 512], DT.bfloat16)
                nc.scalar.activation(
                    out=h1, in_=mega[:, 0:512], func=RELU,
                    bias=sb_b2c, scale=1.0,
                )
                return h1

            def issue_l2relu():
                """relu of l2 -> h2 in SBUF (gates only the next L2)."""
                h2 = h2pool.tile([128, 1024], DT.bfloat16)
                nc.scalar.activation(
                    out=h2, in_=mega[:, 512:1536], func=RELU,
                    bias=sb_b2c, scale=1.0,
                )
                return h2

            def issue_l2(h1):
                nc.tensor.matmul(
                    mega[:, 512:1024], lhsT=w2lo, rhs=h1[0:64, :],
                    start=True, stop=True,
                )
                nc.tensor.matmul(
                    mega[:, 1024:1536], lhsT=w2hi, rhs=h1[64:128, :],
                    start=True, stop=True,
                )

            def g_bank(b):
                return mega[:, 1536 + 512 * b : 2048 + 512 * b]

            def g_banks(i):
                base = (4 * i) % 5
                return [(base + k) % 5 for k in range(4)]

            def issue_l3(i, h2):
                bk = g_banks(i)
                nc.tensor.matmul(
                    g_bank(bk[0]), lhsT=w3a, rhs=h2[:, 0:512],
                    start=True, stop=True,
                )
                nc.tensor.matmul(
                    g_bank(bk[1]), lhsT=w3a, rhs=h2[:, 512:1024],
                    start=True, stop=True,
                )
                nc.tensor.matmul(
                    g_bank(bk[2]), lhsT=w3b, rhs=h2[:, 0:512],
                    start=True, stop=True,
                )
                nc.tensor.matmul(
                    g_bank(bk[3]), lhsT=w3b, rhs=h2[:, 512:1024],
                    start=True, stop=True,
                )

            def dir_reduce(g, racc, cnt, b0, b1):
                """reduce_max of G banks b0,b1 into racc columns."""
                if b1 == b0 + 1:
                    t = cnt[g]
                    cnt[g] += 1
                    nc.vector.reduce_max(
                        out=racc[g][:, t : t + 1],
                        in_=mega[:, 1536 + 512 * b0 : 2560 + 512 * b0],
                        axis=AX.X,
                    )
                else:
                    for b in (b0, b1):
                        t = cnt[g]
                        cnt[g] += 1
                        nc.vector.reduce_max(
                            out=racc[g][:, t : t + 1], in_=g_bank(b), axis=AX.X
                        )

            def issue_drains(i):
                """Drain G(i): slot A (ch 0:128) -> racc; slot B: ACT copy
                (lagged TT) on copy epochs, else DVE reduce into racc2."""
                g = i % 2
                bk = g_banks(i)
                dir_reduce(g, raccs, dcount, bk[0], bk[1])
                if _is_copy(i):
                    gc = gcpool.tile([128, 1024], DT.float16)
                    if bk[3] == bk[2] + 1:
                        nc.scalar.copy(out=gc, in_=mega[:, 1536 + 512 * bk[2] : 2560 + 512 * bk[2]])
                    else:
                        nc.scalar.copy(out=gc[:, 0:512], in_=g_bank(bk[2]))
                        nc.scalar.copy(out=gc[:, 512:1024], in_=g_bank(bk[3]))
                    return gc
                dir_reduce(g, racc2s, d2count, bk[2], bk[3])
                return None

            def issue_tt(i, gc, spare):
                """Lagged running max of epoch i's slot-B copy."""
                g = i % 2
                if rb_cur[g] is None:
                    rb_cur[g] = rb_bufs[g]
                    nc.vector.tensor_copy(out=rb_cur[g], in_=gc)
                    return spare
                nc.vector.tensor_tensor(
                    out=spare, in0=gc, in1=rb_cur[g], op=ALU.max
                )
                old = rb_cur[g]
                rb_cur[g] = spare
                return old

            # ---- main lockstep pipeline: sel(i+2), l2relu(i),
            # prerelu(i+2), L2(i+1), L3(i-1), drains(i-1), lagged TT ----
            h1s, h2s, gcs = {}, {}, {}
            tt_spare = rb_bufs[2]

            issue_sel(0)
            h1s[0] = issue_prerelu()
            issue_l2(h1s[0])
            issue_sel(1)
            h1s[1] = issue_prerelu()

            for i in range(NITER):
                if i + 2 < NITER:
                    issue_sel(i + 2)
                h2s[i] = issue_l2relu()
                if i + 2 < NITER:
                    h1s[i + 2] = issue_prerelu()
                if i + 1 < NITER:
                    issue_l2(h1s[i + 1])
                if i >= 1:
                    issue_l3(i - 1, h2s[i - 1])
                    gc = issue_drains(i - 1)
                    if gc is not None:
                        gcs[i - 1] = gc
                if i >= 2 and (i - 2) in gcs:
                    tt_spare = issue_tt(i - 2, gcs.pop(i - 2), tt_spare)
            issue_l3(NITER - 1, h2s[NITER - 1])
            gc = issue_drains(NITER - 1)
            if gc is not None:
                gcs[NITER - 1] = gc
            for i in (NITER - 2, NITER - 1):
                if i in gcs:
                    tt_spare = issue_tt(i, gcs.pop(i), tt_spare)

            # ---- P per group, batched F-MLP (N=2); pb is (half, group) ----
            pb = pers.tile([128, 2, 2], DT.float16, tag="pb")
            tmp2s = pers.tile([128, 2], DT.float32, tag="tmp2s")
            for g in range(GROUPS_PER_CORE):
                nc.vector.reduce_max(
                    out=pm2s[g][:, 0:1], in_=raccs[g][:, 0 : dcount[g]],
                    axis=AX.X,
                )
                nc.vector.reduce_max(
                    out=pm2s[g][:, 1:2], in_=rb_cur[g], axis=AX.X
                )
                nc.vector.reduce_max(
                    out=tmp2s[:, g : g + 1], in_=racc2s[g][:, 0 : d2count[g]],
                    axis=AX.X,
                )
                nc.vector.tensor_tensor(
                    out=pm2s[g][:, 1:2], in0=pm2s[g][:, 1:2],
                    in1=tmp2s[:, g : g + 1], op=ALU.max,
                )
                nc.vector.tensor_tensor(
                    out=pb[:, :, g], in0=pm2s[g], in1=sb_b3_2, op=ALU.add
                )

            ones2 = sb_v[0:1, 2936:2938]
            y1ps = mega[:, 0:8].rearrange("p (m g) -> p m g", m=4)
            for mm in range(4):
                for kk in range(2):
                    nc.tensor.matmul(
                        y1ps[:, mm, :],
                        lhsT=v1t(kk)[:, mm * 128 : (mm + 1) * 128],
                        rhs=pb[:, kk, :],
                        start=(kk == 0),
                        stop=False,
                    )
                nc.tensor.matmul(
                    y1ps[:, mm, :],
                    lhsT=sb_v[0:1, 2128 + mm * 128 : 2128 + (mm + 1) * 128],
                    rhs=ones2,
                    start=False, stop=True,
                )
            y1 = pers.tile([128, 4, 2], DT.float16, tag="y1")
            nc.vector.tensor_scalar_max(
                out=y1.rearrange("p m g -> p (m g)"), in0=mega[:, 0:8],
                scalar1=0.0,
            )

            y2ps = mega[:, 1024:1028].rearrange("p (m g) -> p m g", m=2)
            for mm in range(2):
                for kk in range(4):
                    nc.tensor.matmul(
                        y2ps[:, mm, :],
                        lhsT=v2t(kk)[:, mm * 128 : (mm + 1) * 128],
                        rhs=y1[:, kk, :],
                        start=(kk == 0),
                        stop=False,
                    )
                nc.tensor.matmul(
                    y2ps[:, mm, :],
                    lhsT=sb_v[0:1, 2640 + mm * 128 : 2640 + (mm + 1) * 128],
                    rhs=ones2,
                    start=False, stop=True,
                )
            y2 = pers.tile([128, 2, 2], DT.float16, tag="y2")
            nc.vector.tensor_scalar_max(
                out=y2.rearrange("p m g -> p (m g)"), in0=mega[:, 1024:1028],
                scalar1=0.0,
            )

            y3ps = mega[0:40, 2048:2050]
            for kk in range(2):
                nc.tensor.matmul(
                    y3ps,
                    lhsT=v3t(kk)[:, 0:40],
                    rhs=y2[:, kk, :],
                    start=(kk == 0),
                    stop=False,
                )
            nc.tensor.matmul(
                y3ps, lhsT=sb_v[0:1, 2896:2936], rhs=ones2,
                start=False, stop=True,
            )
            y3 = pers.tile([40, 2], DT.float32, tag="y3")
            nc.vector.tensor_copy(out=y3, in_=y3ps)
            nc.gpsimd.dma_start(out=y_out[:, :], in_=y3)

    _split_multi_waits(nc)
    return nc


# ---------------------------------------------------------------------------
# Host side
# ---------------------------------------------------------------------------
_NC_CACHE = None


def _get_program():
    global _NC_CACHE
    if _NC_CACHE is None:
        _NC_CACHE = _build_program()
    return _NC_CACHE


def _make_in_maps(inputs):
    X = np.asarray(inputs["X"], F32)
    W1 = np.asarray(inputs["W1"], F32)
    b1 = np.asarray(inputs["b1"], F32)
    W2 = np.asarray(inputs["W2"], F32)
    b2 = np.asarray(inputs["b2"], F32)
    W3 = np.asarray(inputs["W3"], F32)
    b3 = np.asarray(inputs["b3"], F32)
    V1 = np.asarray(inputs["V1"], F32)
    c1 = np.asarray(inputs["c1"], F32)
    V2 = np.asarray(inputs["V2"], F32)
    c2 = np.asarray(inputs["c2"], F32)
    V3 = np.asarray(inputs["V3"], F32)
    c3 = np.asarray(inputs["c3"], F32)

    W1A, W1B = W1[:, :DIN], W1[:, DIN:]
    # sel[jp, it*512 + q*128 + i] = (jp == 4*it + q)
    selblob = np.zeros((JP, (NITER // 2) * 512), F32)
    for jp in range(JP):
        it, q = jp // 4, jp % 4
        selblob[jp, it * 512 + q * 128 : it * 512 + (q + 1) * 128] = 1.0
    selblob = selblob.astype(BF16)

    bigblob = np.zeros((128, 1280), F32)
    # u-rhs halves: ones-row contributes b1 - b2[half] (cancels the fused
    # relu's +b2 bias on the pre region)
    bigblob[0:DIN, 0:64] = W1A.T
    bigblob[DIN, 0:64] = b1 - b2[0:64]
    bigblob[0:DIN, 64:128] = W1A.T
    bigblob[DIN, 64:128] = b1 - b2[64:128]
    bigblob[0:DIN, 128:256] = np.concatenate([W1B.T, W1B.T], axis=1)
    # iden region
    bigblob[:, 256:768] = np.tile(np.eye(M, dtype=F32), (1, 4))
    # packed W2 + W3
    bigblob[0:64, 768:896] = W2.T
    bigblob[64:128, 896:1024] = W2.T
    bigblob[:, 1024:1152] = W3.T[:, 0:128]
    bigblob[:, 1152:1280] = W3.T[:, 128:256]
    bigblob = bigblob.astype(BF16)

    v1t_cols = V1.T.reshape(2, 128, 512).transpose(1, 0, 2).reshape(128, 1024)
    crows = np.zeros((128, 810), F32)
    crows[0, 0:512] = c1
    crows[0, 512:768] = c2
    crows[0, 768:808] = c3
    crows[0, 808:810] = 1.0
    vblob = np.concatenate(
        [v1t_cols,
         V2.T.reshape(4, 128, 256).transpose(1, 0, 2).reshape(128, 1024),
         V3.T.reshape(2, 128, 40).transpose(1, 0, 2).reshape(128, 80),
         crows],
        axis=1,
    ).astype(np.float16)
    cblob = np.zeros((128, 19), F32)
    cblob[:, 17:19] = 1.0
    cblob[:, 1] = b2
    cblob[:, 2:4] = b3.reshape(2, 128).T
    cblob[:, 4:12] = np.repeat(c1.reshape(4, 128).T, 2, axis=1)
    cblob[:, 12:16] = np.repeat(c2.reshape(2, 128).T, 2, axis=1)
    cblob[0:40, 16] = c3

    shared = dict(
        bigb=bigblob, vblob=vblob, cblob=cblob, selb=selblob,
    )

    Xv = X.reshape(B, D, M, DIN)
    in_maps = []
    for c in range(N_CORES):
        xts = np.ones((DIN + 1, GROUPS_PER_CORE, M), F32)
        for gi in range(GROUPS_PER_CORE):
            g = 2 * c + gi
            bb, dd = g // D, g % D
            xts[0:DIN, gi] = Xv[bb, dd].T
        in_maps.append(dict(shared, xt=xts.astype(BF16)))
    return in_maps


def _run(inputs, trace=False):
    nc = _get_program()
    in_maps = _make_in_maps(inputs)
    res = run_bass_kernel_spmd(nc, in_maps, list(range(N_CORES)), trace=trace)
    ys = np.stack([res.results[c]["y"].T for c in range(N_CORES)])  # [8, 2, 40]
    y16 = ys.reshape(B, D, 40)
    out = y16.max(axis=1).astype(F32)
    return out, res


def kernel(**inputs):
    out, _ = _run(inputs, trace=False)
    return out
